# revision 3
# baseline (speedup 1.0000x reference)
"""PoseConsistencyLoss TRN2 kernel v2 (8-core SPMD): pair-min via u/|v| trick.

Math: outputs depend only on per-landmark min_n D^2 (see baseline notes).
For each *pair* of splats (a, b) and landmark c, both
  v = (D^2_a - D^2_b)/2 = c.(b-a) + (|a|^2-|b|^2)/2
  u = (D^2_a + D^2_b)/2 = -c.(a+b) + (|a|^2+|b|^2)/2 + |c|^2
are LINEAR in per-pair features, so the PE can compute them: stream V into
PSUM, ACT overwrites a second PSUM bank with |V| (Abs), then the PE
*accumulates* the (-u)-stream on top (start=False):
  psW = |v| - u = -min(D^2_a, D^2_b).
This halves the volume the PSUM consumers must reduce: DVE just runs wide
column-max TRs over psW (max of -pairmin = -minsq).

PE speed: TRN2 streams 512-col matmuls at 216ns only when the stationary
row count is 128 (measured: K=128 triggers the fast PE clock, K=64 sustains,
K<=24 decays to 427ns). All matmuls here use K=128 (zero-padded), bf16.

Precision: bf16 3-way limb splits (hi/mid/lo) of c and of pair features,
keeping product groups {hh, hm, mh, mm, hl, lh} -> error ~1e-5 absolute on
D^2. |c|^2 is folded into the (-u)-stream as 3 csq-limb rows x 1.0.

Sharding: splats split across 8 cores (8192 -> 4096 pairs each); every core
covers all 2048 landmarks; host takes the 8-way min + masked reduction.
"""

import os
import sys
import time

sys.path.insert(0, "/opt/trn_rl_repo")

import numpy as np
import ml_dtypes

import concourse.bass as bass
import concourse.bacc as bacc
import concourse.tile as tile
from concourse import mybir
from concourse.bass_utils import run_bass_kernel_spmd

# Disk-cache NEFF compiles.
import concourse.bass_utils as _bu
import concourse.bass2jax as _b2j

_orig_compile_bir = _bu.compile_bir_kernel
_NEFF_CACHE = os.environ.get("BASS_NEFF_CACHE_DIR", "/tmp/bass_neff_cache")


def _cached_compile_bir(bir_json, tmpdir, neff_name="file.neff"):
    import hashlib
    import shutil

    h = hashlib.sha256(bir_json).hexdigest()[:24]
    os.makedirs(_NEFF_CACHE, exist_ok=True)
    cpath = os.path.join(_NEFF_CACHE, f"{h}_{neff_name}")
    out = os.path.join(tmpdir, neff_name)
    if os.path.exists(cpath):
        shutil.copyfile(cpath, out)
        return out
    p = _orig_compile_bir(bir_json, tmpdir, neff_name=neff_name)
    try:
        shutil.copyfile(p, cpath)
    except OSError:
        pass
    return p


_bu.compile_bir_kernel = _cached_compile_bir
_b2j.compile_bir_kernel = _cached_compile_bir

F32 = mybir.dt.float32
BF16 = mybir.dt.bfloat16
AF = mybir.ActivationFunctionType
ALU = mybir.AluOpType
AX = mybir.AxisListType

BF = ml_dtypes.bfloat16

CFG = dict(
    n_cores=8,
    s_per_core=8192,
    m_total=2048,
    chunk=1024,  # pair-columns per psum chunk
)

KROWS = 128  # stationary rows; K=128 everywhere (mixing 64/128-row matmuls
             # adds ~190ns/chunk, and garbage operands stall the clock boost)
KREAL = 128
KP = 32  # host sends rows [0:32]; device zero-pads [32:128] in parallel


def build(cfg):
    C = cfg["n_cores"]
    S = cfg["s_per_core"]
    M = cfg["m_total"]
    CH = cfg["chunk"]
    P = S // 2              # pair columns per core
    NCH = P // CH           # psum chunks per landmark tile
    MT = M // 128           # landmark tiles

    nc = bacc.Bacc("TRN2", target_bir_lowering=False, debug=False, num_devices=C)

    featv_d = nc.dram_tensor("featv", [KP, P], BF16, kind="ExternalInput")
    featw_d = nc.dram_tensor("featw", [KP, P], BF16, kind="ExternalInput")
    lhsv_d = nc.dram_tensor("lhsv", [KP, M], BF16, kind="ExternalInput")
    lhsw_d = nc.dram_tensor("lhsw", [KP, M], BF16, kind="ExternalInput")
    out_d = nc.dram_tensor("negmin", [128, MT], F32, kind="ExternalOutput")

    with tile.TileContext(nc) as tc:
        with (
            tc.tile_pool(name="persist", bufs=1) as persist,
        ):
            # Zero-pad rows [32:128] while the row-[0:32] DMAs stream in.
            # Non-zero partition bases are limited to 32 partitions per op;
            # uint32 bitcast halves the free-size cost; spread the 12 ops
            # over DVE/ACT/GPSIMD so they run in parallel (~5.5us each).
            lhsv = persist.tile([KROWS, M], BF16, tag="lhsv")
            lhsw = persist.tile([KROWS, M], BF16, tag="lhsw")
            featv = persist.tile([KROWS, P], BF16, tag="featv")
            featw = persist.tile([KROWS, P], BF16, tag="featw")
            # Pad rows [32:128] with zeros, column-chunked and scheduled so
            # chunk-0 pads land first: DVE takes featv ch0/1, ACT featw ch0/1,
            # GPSIMD (idle during compute) takes lhs + the ch2/3 tails in an
            # order that stays ahead of the compute loop.
            U32 = mybir.dt.uint32
            BASES = (KP, KP + 32, KP + 64)

            def pad(eng, t, b, c0, c1):
                if eng is nc.scalar:
                    eng.memzero(t[b : b + 32, c0:c1])
                else:
                    eng.memset(t[b : b + 32, c0:c1].bitcast(U32), 0)

            for b in BASES:
                pad(nc.vector, featv, b, 0, CH)
                pad(nc.scalar, featw, b, 0, CH)
            for b in BASES:
                pad(nc.vector, featv, b, CH, 2 * CH)
                pad(nc.scalar, featw, b, CH, 2 * CH)
            for b in BASES:
                pad(nc.gpsimd, lhsv, b, 0, M)
                pad(nc.gpsimd, lhsw, b, 0, M)
            for ch in (2, 3):
                for b in BASES:
                    pad(nc.gpsimd, featv, b, ch * CH, (ch + 1) * CH)
                    pad(nc.gpsimd, featw, b, ch * CH, (ch + 1) * CH)
            nc.sync.dma_start(lhsv[0:KP, :], lhsv_d[:])
            nc.sync.dma_start(lhsw[0:KP, :], lhsw_d[:])
            for ch in range(NCH):
                sl = slice(ch * CH, (ch + 1) * CH)
                nc.sync.dma_start(featv[0:KP, sl], featv_d[:, sl])
                nc.sync.dma_start(featw[0:KP, sl], featw_d[:, sl])

            wmax = persist.tile([128, MT * NCH], F32, tag="wmax")
            wfin = persist.tile([128, MT], F32, tag="wfin")

            # 3 rotating chunk tiles (v -> |v| in-place -> -pairmin) + 1 dummy
            # target tile for clock-keeper matmuls: 4 * 4KB = all 8 PSUM banks.
            pp = tc.alloc_tile_pool(name="psum", bufs=3, space="PSUM")
            ppd = tc.alloc_tile_pool(name="psdum", bufs=1, space="PSUM")
            dummy = ppd.tile([128, CH], F32, tag="dum")
            DUMMIES = cfg.get("dummies", 2)

            def emit_v(ps, mt, ch):
                lv = lhsv[0:KREAL, mt * 128 : (mt + 1) * 128]
                for h in range(CH // 512):
                    off = ch * CH + h * 512
                    nc.tensor.matmul(
                        ps[:, h * 512 : h * 512 + 512], lv,
                        featv[0:KREAL, off : off + 512], start=True, stop=True,
                    )
                # |v| in place: WAR+WAW edges order the later accumulate
                nc.scalar.activation(ps[:], ps[:], AF.Abs)

            def emit_w(ps, mt, ch, ndum):
                lw = lhsw[0:KREAL, mt * 128 : (mt + 1) * 128]
                for h in range(CH // 512):
                    off = ch * CH + h * 512
                    nc.tensor.matmul(
                        ps[:, h * 512 : h * 512 + 512], lw,
                        featw[0:KREAL, off : off + 512], start=False, stop=True,
                        skip_group_check=True,
                    )
                # clock keepers: K=128 matmuls into the dummy tile keep the
                # PE at its boosted clock through consumer-bound stretches
                for dj in range(ndum):
                    half = (dj % 2) * 512
                    nc.tensor.matmul(
                        dummy[:, half : half + 512], lhsv[:, 0:128],
                        featv[:, 0:512], start=True, stop=True,
                    )

            def emit_tr(ps, mt, ch):
                nc.vector.tensor_reduce(
                    wmax[:, mt * NCH + ch : mt * NCH + ch + 1], ps[:],
                    AX.X, ALU.max,
                )

            # 3-stage software pipeline: v_k | w_{k-1} | tr_{k-2}. The TR
            # lags the accumulate by a full chunk so the DVE never waits on
            # the w-matmul semaphore (back-to-back TRs).
            chunks = [(mt, ch) for mt in range(MT) for ch in range(NCH)]
            tiles = {}
            k = 0
            for k, (mt, ch) in enumerate(chunks):
                ps_k = pp.tile([128, CH], F32, tag="ps")
                tiles[k] = ps_k
                emit_v(tiles[k], mt, ch)
                if k >= 1:
                    pm, pc = chunks[k - 1]
                    emit_w(tiles[k - 1], pm, pc, 1)
                if k >= 2:
                    qm, qc = chunks[k - 2]
                    emit_tr(tiles.pop(k - 2), qm, qc)
            emit_w(tiles[k], *chunks[k], 1)
            emit_tr(tiles.pop(k - 1), *chunks[k - 1])
            emit_tr(tiles.pop(k), *chunks[k])
            # combine chunk maxes: [128, MT, NCH] -> [128, MT]
            nc.vector.tensor_reduce(
                wfin[:], wmax[:].rearrange("p (a b) -> p a b", a=MT), AX.X, ALU.max
            )
            ppd.release()
            pp.release()
            nc.sync.dma_start(out_d[:], wfin[:])

    nc.compile()
    return nc


def _limb3(x):
    """bf16 3-way limb split of fp32 array: x ~= hi + mid + lo."""
    x = np.ascontiguousarray(x, np.float32)
    hi = x.astype(BF)
    r1 = x - hi.astype(np.float32)
    mid = r1.astype(BF)
    lo = (r1 - mid.astype(np.float32)).astype(BF)
    return hi, mid, lo


def _landmarks_cam(camera_pose, landmarks_3d):
    pose = np.asarray(camera_pose, np.float32)
    lm = np.asarray(landmarks_3d, np.float32)
    hom = np.concatenate([lm, np.ones((lm.shape[0], 1), np.float32)], axis=1)
    return (pose @ hom.T).T[:, :3].astype(np.float32)  # [M, 3]


def _fill_groups(dst, hi, mid, lo):
    """Rows 0-17: product groups {hh, hm, mh, mm, hl, lh} of a 3-dim factor."""
    dst[0:3] = hi
    dst[3:6] = mid
    dst[6:9] = hi
    dst[9:12] = mid
    dst[12:15] = lo
    dst[15:18] = hi


def _fill_groups_lhs(dst, hi, mid, lo):
    dst[0:3] = hi
    dst[3:6] = hi
    dst[6:9] = mid
    dst[9:12] = mid
    dst[12:15] = hi
    dst[15:18] = lo


def make_in_maps(cfg, splat_positions, camera_pose, landmarks_3d):
    C = cfg["n_cores"]
    S = cfg["s_per_core"]
    M = cfg["m_total"]
    P = S // 2
    sp = np.ascontiguousarray(np.asarray(splat_positions, np.float32))
    cam = _landmarks_cam(camera_pose, landmarks_3d)  # [M, 3]
    csq = np.sum(cam**2, axis=1, dtype=np.float32)  # [M]

    cT = cam.T  # [3, M]
    c_hi, c_mid, c_lo = _limb3(cT)
    q_hi, q_mid, q_lo = _limb3(-csq)

    lhsv = np.zeros((KP, M), BF)
    _fill_groups_lhs(lhsv, c_hi, c_mid, c_lo)
    lhsv[18:21] = np.float32(1.0)

    lhsw = np.zeros((KP, M), BF)
    _fill_groups_lhs(lhsw, c_hi, c_mid, c_lo)
    lhsw[18:21] = np.float32(1.0)
    lhsw[21] = q_hi
    lhsw[22] = q_mid
    lhsw[23] = q_lo

    maps = []
    for c in range(C):
        shard = sp[c * S : (c + 1) * S]  # [S, 3]
        a = shard[0::2]  # [P, 3]
        b = shard[1::2]
        sd = (b - a).T  # [3, P]  (v = c.(b-a) + sqdiff)
        ss = (a + b).T  # [3, P]  (-u = c.(a+b) - sqsum - csq)
        na = np.sum(a * a, axis=1, dtype=np.float32)
        nb = np.sum(b * b, axis=1, dtype=np.float32)
        sqdiff = (na - nb) * np.float32(0.5)  # [P]
        sqsum = (na + nb) * np.float32(-0.5)  # [P] (negated for -u)

        sd_hi, sd_mid, sd_lo = _limb3(sd)
        ss_hi, ss_mid, ss_lo = _limb3(ss)
        qd_hi, qd_mid, qd_lo = _limb3(sqdiff)
        qs_hi, qs_mid, qs_lo = _limb3(sqsum)

        featv = np.zeros((KP, P), BF)
        _fill_groups(featv, sd_hi, sd_mid, sd_lo)
        featv[18] = qd_hi
        featv[19] = qd_mid
        featv[20] = qd_lo

        featw = np.zeros((KP, P), BF)
        _fill_groups(featw, ss_hi, ss_mid, ss_lo)
        featw[18] = qs_hi
        featw[19] = qs_mid
        featw[20] = qs_lo
        featw[21:24] = np.float32(1.0)

        maps.append(
            {"featv": featv, "featw": featw, "lhsv": lhsv, "lhsw": lhsw}
        )
    return maps


_COMPILED = None


def _get_compiled():
    global _COMPILED
    if _COMPILED is None:
        _COMPILED = build(CFG)
    return _COMPILED


def kernel(
    splat_positions,
    camera_pose,
    landmarks_3d,
    landmarks_2d=None,
    camera_intrinsics=None,
    **_unused,
):
    nc = _get_compiled()
    in_maps = make_in_maps(CFG, splat_positions, camera_pose, landmarks_3d)
    core_ids = list(range(CFG["n_cores"]))
    try:
        res = run_bass_kernel_spmd(nc, in_maps, core_ids)
    except Exception:
        time.sleep(5.0)
        res = run_bass_kernel_spmd(nc, in_maps, core_ids)

    # negmin[p, mt] = -min over this core's splats of D^2 for landmark mt*128+p
    parts = np.stack([r["negmin"] for r in res.results], axis=0)  # [C, 128, MT]
    negmin = parts.max(axis=0)  # [128, MT]
    msq = np.maximum(-negmin, np.float32(0.0))  # [128, MT]
    MT = CFG["m_total"] // 128
    msq = msq.T.reshape(-1)  # m = mt*128 + p -> index [mt, p]
    d = np.sqrt(msq)
    valid = d < np.float32(1.0)
    num = np.int32(valid.sum())
    loss = np.float32(
        (msq * valid).sum(dtype=np.float32)
        / max(np.float32(3.0) * np.float32(num), np.float32(1.0))
    )
    meand = np.float32(
        (d * valid).sum(dtype=np.float32) / max(np.float32(num), np.float32(1.0))
    )
    return loss, num, meand


if __name__ == "__main__":
    build(CFG)
    print("build ok")


# revision 4
# speedup vs baseline: 1.2513x; 1.2513x over previous
"""PoseConsistencyLoss TRN2 kernel v2 (8-core SPMD): pair-min via u/|v| trick.

Math: outputs depend only on per-landmark min_n D^2 (see baseline notes).
For each *pair* of splats (a, b) and landmark c, both
  v = (D^2_a - D^2_b)/2 = c.(b-a) + (|a|^2-|b|^2)/2
  u = (D^2_a + D^2_b)/2 = -c.(a+b) + (|a|^2+|b|^2)/2 + |c|^2
are LINEAR in per-pair features, so the PE can compute them: stream V into
PSUM, ACT overwrites a second PSUM bank with |V| (Abs), then the PE
*accumulates* the (-u)-stream on top (start=False):
  psW = |v| - u = -min(D^2_a, D^2_b).
This halves the volume the PSUM consumers must reduce: DVE just runs wide
column-max TRs over psW (max of -pairmin = -minsq).

PE speed: TRN2 streams 512-col matmuls at 216ns only when the stationary
row count is 128 (measured: K=128 triggers the fast PE clock, K=64 sustains,
K<=24 decays to 427ns). All matmuls here use K=128 (zero-padded), bf16.

Precision: bf16 3-way limb splits (hi/mid/lo) of c and of pair features,
keeping product groups {hh, hm, mh, mm, hl, lh} -> error ~1e-5 absolute on
D^2. |c|^2 is folded into the (-u)-stream as 3 csq-limb rows x 1.0.

Sharding: splats split across 8 cores (8192 -> 4096 pairs each); every core
covers all 2048 landmarks; host takes the 8-way min + masked reduction.
"""

import os
import sys
import time

sys.path.insert(0, "/opt/trn_rl_repo")

import numpy as np
import ml_dtypes

import concourse.bass as bass
import concourse.bacc as bacc
import concourse.tile as tile
from concourse import mybir
from concourse.bass_utils import run_bass_kernel_spmd

# Disk-cache NEFF compiles.
import concourse.bass_utils as _bu
import concourse.bass2jax as _b2j

_orig_compile_bir = _bu.compile_bir_kernel
_NEFF_CACHE = os.environ.get("BASS_NEFF_CACHE_DIR", "/tmp/bass_neff_cache")


def _cached_compile_bir(bir_json, tmpdir, neff_name="file.neff"):
    import hashlib
    import shutil

    h = hashlib.sha256(bir_json).hexdigest()[:24]
    os.makedirs(_NEFF_CACHE, exist_ok=True)
    cpath = os.path.join(_NEFF_CACHE, f"{h}_{neff_name}")
    out = os.path.join(tmpdir, neff_name)
    if os.path.exists(cpath):
        shutil.copyfile(cpath, out)
        return out
    p = _orig_compile_bir(bir_json, tmpdir, neff_name=neff_name)
    try:
        shutil.copyfile(p, cpath)
    except OSError:
        pass
    return p


_bu.compile_bir_kernel = _cached_compile_bir
_b2j.compile_bir_kernel = _cached_compile_bir

F32 = mybir.dt.float32
BF16 = mybir.dt.bfloat16
AF = mybir.ActivationFunctionType
ALU = mybir.AluOpType
AX = mybir.AxisListType

BF = ml_dtypes.bfloat16

CFG = dict(
    n_cores=8,
    s_per_core=8192,
    m_total=2048,
    chunk=1024,  # pair-columns per psum chunk
)

KROWS = 128  # stationary rows; K=128 everywhere (mixing 64/128-row matmuls
             # adds ~190ns/chunk, and garbage operands stall the clock boost)
KREAL = 128
KP = 32  # host sends rows [0:32]; device zero-pads [32:128] in parallel


def build(cfg):
    C = cfg["n_cores"]
    S = cfg["s_per_core"]
    M = cfg["m_total"]
    CH = cfg["chunk"]
    P = S // 2              # pair columns per core
    NCH = P // CH           # psum chunks per landmark tile
    MT = M // 128           # landmark tiles

    nc = bacc.Bacc("TRN2", target_bir_lowering=False, debug=False, num_devices=C)

    featv_d = nc.dram_tensor("featv", [KP, P], BF16, kind="ExternalInput")
    featw_d = nc.dram_tensor("featw", [KP, P], BF16, kind="ExternalInput")
    lhsv_d = nc.dram_tensor("lhsv", [KP, M], BF16, kind="ExternalInput")
    lhsw_d = nc.dram_tensor("lhsw", [KP, M], BF16, kind="ExternalInput")
    out_d = nc.dram_tensor("negmin", [128, MT], F32, kind="ExternalOutput")

    with tile.TileContext(nc) as tc:
        with (
            tc.tile_pool(name="persist", bufs=1) as persist,
        ):
            # Zero-pad rows [32:128] while the row-[0:32] DMAs stream in.
            # Non-zero partition bases are limited to 32 partitions per op;
            # uint32 bitcast halves the free-size cost; spread the 12 ops
            # over DVE/ACT/GPSIMD so they run in parallel (~5.5us each).
            lhsv = persist.tile([KROWS, M], BF16, tag="lhsv")
            lhsw = persist.tile([KROWS, M], BF16, tag="lhsw")
            featv = persist.tile([KROWS, P], BF16, tag="featv")
            featw = persist.tile([KROWS, P], BF16, tag="featw")
            # Pad rows [32:128] with zeros, column-chunked and scheduled so
            # chunk-0 pads land first: DVE takes featv ch0/1, ACT featw ch0/1,
            # GPSIMD (idle during compute) takes lhs + the ch2/3 tails in an
            # order that stays ahead of the compute loop.
            U32 = mybir.dt.uint32
            BASES = (KP, KP + 32, KP + 64)

            def pad(eng, t, b, c0, c1):
                if eng is nc.scalar:
                    eng.memzero(t[b : b + 32, c0:c1])
                else:
                    eng.memset(t[b : b + 32, c0:c1].bitcast(U32), 0)

            # chunk-0 pads spread over all three engines so featv/featw ch0
            # are ready fastest; GPSIMD takes lhs + the later-chunk tails
            pad(nc.vector, featv, KP, 0, CH)
            pad(nc.vector, featv, KP + 32, 0, CH)
            pad(nc.scalar, featw, KP, 0, CH)
            pad(nc.scalar, featw, KP + 32, 0, CH)
            pad(nc.gpsimd, lhsv, KP, 0, M)
            pad(nc.gpsimd, lhsv, KP + 32, 0, M)
            pad(nc.gpsimd, lhsv, KP + 64, 0, M)
            pad(nc.vector, featv, KP + 64, 0, CH)
            pad(nc.scalar, featw, KP + 64, 0, CH)
            for b in BASES:
                pad(nc.vector, featv, b, CH, 2 * CH)
                pad(nc.scalar, featw, b, CH, 2 * CH)
            for b in BASES:
                pad(nc.gpsimd, lhsw, b, 0, M)
            for ch in (2, 3):
                for b in BASES:
                    pad(nc.gpsimd, featv, b, ch * CH, (ch + 1) * CH)
                    pad(nc.gpsimd, featw, b, ch * CH, (ch + 1) * CH)
            # DMA order: what chunk 0 needs first
            nc.sync.dma_start(lhsv[0:KP, :], lhsv_d[:])
            c0 = slice(0, CH)
            nc.sync.dma_start(featv[0:KP, c0], featv_d[:, c0])
            nc.sync.dma_start(featw[0:KP, c0], featw_d[:, c0])
            nc.sync.dma_start(lhsw[0:KP, :], lhsw_d[:])
            for ch in range(1, NCH):
                sl = slice(ch * CH, (ch + 1) * CH)
                nc.sync.dma_start(featv[0:KP, sl], featv_d[:, sl])
                nc.sync.dma_start(featw[0:KP, sl], featw_d[:, sl])

            wmax = persist.tile([128, MT * NCH], F32, tag="wmax")
            wfin = persist.tile([128, MT], F32, tag="wfin")

            # 3 rotating chunk tiles (v -> |v| in-place -> -pairmin) + 1 dummy
            # target tile for clock-keeper matmuls: 4 * 4KB = all 8 PSUM banks.
            pp = tc.alloc_tile_pool(name="psum", bufs=3, space="PSUM")
            ppd = tc.alloc_tile_pool(name="psdum", bufs=1, space="PSUM")
            dummy = ppd.tile([128, CH], F32, tag="dum")
            DUMMIES = cfg.get("dummies", 2)

            def emit_v(ps, mt, ch):
                lv = lhsv[0:KREAL, mt * 128 : (mt + 1) * 128]
                for h in range(CH // 512):
                    off = ch * CH + h * 512
                    nc.tensor.matmul(
                        ps[:, h * 512 : h * 512 + 512], lv,
                        featv[0:KREAL, off : off + 512], start=True, stop=True,
                    )
                # |v| in place: WAR+WAW edges order the later accumulate
                nc.scalar.activation(ps[:], ps[:], AF.Abs)

            def emit_w(ps, mt, ch, ndum):
                lw = lhsw[0:KREAL, mt * 128 : (mt + 1) * 128]
                for h in range(CH // 512):
                    off = ch * CH + h * 512
                    nc.tensor.matmul(
                        ps[:, h * 512 : h * 512 + 512], lw,
                        featw[0:KREAL, off : off + 512], start=False, stop=True,
                        skip_group_check=True,
                    )
                # clock keepers: K=128 matmuls into the dummy tile keep the
                # PE at its boosted clock through consumer-bound stretches
                for dj in range(ndum):
                    half = (dj % 2) * 512
                    nc.tensor.matmul(
                        dummy[:, half : half + 512], lhsv[:, 0:128],
                        featv[:, 0:512], start=True, stop=True,
                    )

            def emit_tr(ps, mt, ch):
                nc.vector.tensor_reduce(
                    wmax[:, mt * NCH + ch : mt * NCH + ch + 1], ps[:],
                    AX.X, ALU.max,
                )

            # 3-stage software pipeline: v_k | w_{k-1} | tr_{k-2}. The TR
            # lags the accumulate by a full chunk so the DVE never waits on
            # the w-matmul semaphore (back-to-back TRs).
            chunks = [(mt, ch) for mt in range(MT) for ch in range(NCH)]
            # pre-ramp: boost the PE clock during the DMA wait (reads only
            # lhsv, which lands first; output never read)
            for r in range(12):
                nc.tensor.matmul(
                    dummy[:, 0:512], lhsv[:, 0:128],
                    lhsv[:, (r % 3) * 512 : (r % 3) * 512 + 512],
                    start=True, stop=True,
                )
            tiles = {}
            k = 0
            for k, (mt, ch) in enumerate(chunks):
                ps_k = pp.tile([128, CH], F32, tag="ps")
                tiles[k] = ps_k
                emit_v(tiles[k], mt, ch)
                if k >= 1:
                    pm, pc = chunks[k - 1]
                    emit_w(tiles[k - 1], pm, pc, 1)
                if k >= 2:
                    qm, qc = chunks[k - 2]
                    emit_tr(tiles.pop(k - 2), qm, qc)
            emit_w(tiles[k], *chunks[k], 1)
            emit_tr(tiles.pop(k - 1), *chunks[k - 1])
            emit_tr(tiles.pop(k), *chunks[k])
            # combine chunk maxes: [128, MT, NCH] -> [128, MT]
            nc.vector.tensor_reduce(
                wfin[:], wmax[:].rearrange("p (a b) -> p a b", a=MT), AX.X, ALU.max
            )
            ppd.release()
            pp.release()
            nc.sync.dma_start(out_d[:], wfin[:])

    nc.compile()
    return nc


def _limb3(x):
    """bf16 3-way limb split of fp32 array: x ~= hi + mid + lo."""
    x = np.ascontiguousarray(x, np.float32)
    hi = x.astype(BF)
    r1 = x - hi.astype(np.float32)
    mid = r1.astype(BF)
    lo = (r1 - mid.astype(np.float32)).astype(BF)
    return hi, mid, lo


def _landmarks_cam(camera_pose, landmarks_3d):
    pose = np.asarray(camera_pose, np.float32)
    lm = np.asarray(landmarks_3d, np.float32)
    hom = np.concatenate([lm, np.ones((lm.shape[0], 1), np.float32)], axis=1)
    return (pose @ hom.T).T[:, :3].astype(np.float32)  # [M, 3]


def _fill_groups(dst, hi, mid, lo):
    """Rows 0-17: product groups {hh, hm, mh, mm, hl, lh} of a 3-dim factor."""
    dst[0:3] = hi
    dst[3:6] = mid
    dst[6:9] = hi
    dst[9:12] = mid
    dst[12:15] = lo
    dst[15:18] = hi


def _fill_groups_lhs(dst, hi, mid, lo):
    dst[0:3] = hi
    dst[3:6] = hi
    dst[6:9] = mid
    dst[9:12] = mid
    dst[12:15] = hi
    dst[15:18] = lo


def make_in_maps(cfg, splat_positions, camera_pose, landmarks_3d):
    C = cfg["n_cores"]
    S = cfg["s_per_core"]
    M = cfg["m_total"]
    P = S // 2
    sp = np.ascontiguousarray(np.asarray(splat_positions, np.float32))
    cam = _landmarks_cam(camera_pose, landmarks_3d)  # [M, 3]
    csq = np.sum(cam**2, axis=1, dtype=np.float32)  # [M]

    cT = cam.T  # [3, M]
    c_hi, c_mid, c_lo = _limb3(cT)
    q_hi, q_mid, q_lo = _limb3(-csq)

    lhsv = np.zeros((KP, M), BF)
    _fill_groups_lhs(lhsv, c_hi, c_mid, c_lo)
    lhsv[18:21] = np.float32(1.0)

    lhsw = np.zeros((KP, M), BF)
    _fill_groups_lhs(lhsw, c_hi, c_mid, c_lo)
    lhsw[18:21] = np.float32(1.0)
    lhsw[21] = q_hi
    lhsw[22] = q_mid
    lhsw[23] = q_lo

    maps = []
    for c in range(C):
        shard = sp[c * S : (c + 1) * S]  # [S, 3]
        a = shard[0::2]  # [P, 3]
        b = shard[1::2]
        sd = (b - a).T  # [3, P]  (v = c.(b-a) + sqdiff)
        ss = (a + b).T  # [3, P]  (-u = c.(a+b) - sqsum - csq)
        na = np.sum(a * a, axis=1, dtype=np.float32)
        nb = np.sum(b * b, axis=1, dtype=np.float32)
        sqdiff = (na - nb) * np.float32(0.5)  # [P]
        sqsum = (na + nb) * np.float32(-0.5)  # [P] (negated for -u)

        sd_hi, sd_mid, sd_lo = _limb3(sd)
        ss_hi, ss_mid, ss_lo = _limb3(ss)
        qd_hi, qd_mid, qd_lo = _limb3(sqdiff)
        qs_hi, qs_mid, qs_lo = _limb3(sqsum)

        featv = np.zeros((KP, P), BF)
        _fill_groups(featv, sd_hi, sd_mid, sd_lo)
        featv[18] = qd_hi
        featv[19] = qd_mid
        featv[20] = qd_lo

        featw = np.zeros((KP, P), BF)
        _fill_groups(featw, ss_hi, ss_mid, ss_lo)
        featw[18] = qs_hi
        featw[19] = qs_mid
        featw[20] = qs_lo
        featw[21:24] = np.float32(1.0)

        maps.append(
            {"featv": featv, "featw": featw, "lhsv": lhsv, "lhsw": lhsw}
        )
    return maps


_COMPILED = None


def _get_compiled():
    global _COMPILED
    if _COMPILED is None:
        _COMPILED = build(CFG)
    return _COMPILED


def kernel(
    splat_positions,
    camera_pose,
    landmarks_3d,
    landmarks_2d=None,
    camera_intrinsics=None,
    **_unused,
):
    nc = _get_compiled()
    in_maps = make_in_maps(CFG, splat_positions, camera_pose, landmarks_3d)
    core_ids = list(range(CFG["n_cores"]))
    try:
        res = run_bass_kernel_spmd(nc, in_maps, core_ids)
    except Exception:
        time.sleep(5.0)
        res = run_bass_kernel_spmd(nc, in_maps, core_ids)

    # negmin[p, mt] = -min over this core's splats of D^2 for landmark mt*128+p
    parts = np.stack([r["negmin"] for r in res.results], axis=0)  # [C, 128, MT]
    negmin = parts.max(axis=0)  # [128, MT]
    msq = np.maximum(-negmin, np.float32(0.0))  # [128, MT]
    MT = CFG["m_total"] // 128
    msq = msq.T.reshape(-1)  # m = mt*128 + p -> index [mt, p]
    d = np.sqrt(msq)
    valid = d < np.float32(1.0)
    num = np.int32(valid.sum())
    loss = np.float32(
        (msq * valid).sum(dtype=np.float32)
        / max(np.float32(3.0) * np.float32(num), np.float32(1.0))
    )
    meand = np.float32(
        (d * valid).sum(dtype=np.float32) / max(np.float32(num), np.float32(1.0))
    )
    return loss, num, meand


if __name__ == "__main__":
    build(CFG)
    print("build ok")


# revision 5
# speedup vs baseline: 1.2549x; 1.0029x over previous
"""PoseConsistencyLoss TRN2 kernel v2 (8-core SPMD): pair-min via u/|v| trick.

Math: outputs depend only on per-landmark min_n D^2 (see baseline notes).
For each *pair* of splats (a, b) and landmark c, both
  v = (D^2_a - D^2_b)/2 = c.(b-a) + (|a|^2-|b|^2)/2
  u = (D^2_a + D^2_b)/2 = -c.(a+b) + (|a|^2+|b|^2)/2 + |c|^2
are LINEAR in per-pair features, so the PE can compute them: stream V into
PSUM, ACT overwrites a second PSUM bank with |V| (Abs), then the PE
*accumulates* the (-u)-stream on top (start=False):
  psW = |v| - u = -min(D^2_a, D^2_b).
This halves the volume the PSUM consumers must reduce: DVE just runs wide
column-max TRs over psW (max of -pairmin = -minsq).

PE speed: TRN2 streams 512-col matmuls at 216ns only when the stationary
row count is 128 (measured: K=128 triggers the fast PE clock, K=64 sustains,
K<=24 decays to 427ns). All matmuls here use K=128 (zero-padded), bf16.

Precision: bf16 3-way limb splits (hi/mid/lo) of c and of pair features,
keeping product groups {hh, hm, mh, mm, hl, lh} -> error ~1e-5 absolute on
D^2. |c|^2 is folded into the (-u)-stream as 3 csq-limb rows x 1.0.

Sharding: splats split across 8 cores (8192 -> 4096 pairs each); every core
covers all 2048 landmarks; host takes the 8-way min + masked reduction.
"""

import os
import sys
import time

sys.path.insert(0, "/opt/trn_rl_repo")

import numpy as np
import ml_dtypes

import concourse.bass as bass
import concourse.bacc as bacc
import concourse.tile as tile
from concourse import mybir
from concourse.bass_utils import run_bass_kernel_spmd

# Disk-cache NEFF compiles.
import concourse.bass_utils as _bu
import concourse.bass2jax as _b2j

_orig_compile_bir = _bu.compile_bir_kernel
_NEFF_CACHE = os.environ.get("BASS_NEFF_CACHE_DIR", "/tmp/bass_neff_cache")


def _cached_compile_bir(bir_json, tmpdir, neff_name="file.neff"):
    import hashlib
    import shutil

    h = hashlib.sha256(bir_json).hexdigest()[:24]
    os.makedirs(_NEFF_CACHE, exist_ok=True)
    cpath = os.path.join(_NEFF_CACHE, f"{h}_{neff_name}")
    out = os.path.join(tmpdir, neff_name)
    if os.path.exists(cpath):
        shutil.copyfile(cpath, out)
        return out
    p = _orig_compile_bir(bir_json, tmpdir, neff_name=neff_name)
    try:
        shutil.copyfile(p, cpath)
    except OSError:
        pass
    return p


_bu.compile_bir_kernel = _cached_compile_bir
_b2j.compile_bir_kernel = _cached_compile_bir

F32 = mybir.dt.float32
BF16 = mybir.dt.bfloat16
AF = mybir.ActivationFunctionType
ALU = mybir.AluOpType
AX = mybir.AxisListType

BF = ml_dtypes.bfloat16

CFG = dict(
    n_cores=8,
    s_per_core=8192,
    m_total=2048,
    chunk=1024,  # pair-columns per psum chunk
)

KROWS = 128  # stationary rows; K=128 everywhere (mixing 64/128-row matmuls
             # adds ~190ns/chunk, and garbage operands stall the clock boost)
KREAL = 128
KP = 32  # host sends rows [0:32]; device zero-pads [32:128] in parallel


def build(cfg):
    C = cfg["n_cores"]
    S = cfg["s_per_core"]
    M = cfg["m_total"]
    CH = cfg["chunk"]
    P = S // 2              # pair columns per core
    NCH = P // CH           # psum chunks per landmark tile
    MT = M // 128           # landmark tiles

    nc = bacc.Bacc("TRN2", target_bir_lowering=False, debug=False, num_devices=C)

    featv_d = nc.dram_tensor("featv", [KP, P], BF16, kind="ExternalInput")
    featw_d = nc.dram_tensor("featw", [KP, P], BF16, kind="ExternalInput")
    lhsv_d = nc.dram_tensor("lhsv", [KP, M], BF16, kind="ExternalInput")
    lhsw_d = nc.dram_tensor("lhsw", [KP, M], BF16, kind="ExternalInput")
    out_d = nc.dram_tensor("negmin", [128, MT], F32, kind="ExternalOutput")

    with tile.TileContext(nc) as tc:
        with (
            tc.tile_pool(name="persist", bufs=1) as persist,
        ):
            # Zero-pad rows [32:128] while the row-[0:32] DMAs stream in.
            # Non-zero partition bases are limited to 32 partitions per op;
            # uint32 bitcast halves the free-size cost; spread the 12 ops
            # over DVE/ACT/GPSIMD so they run in parallel (~5.5us each).
            lhsv = persist.tile([KROWS, M], BF16, tag="lhsv")
            lhsw = persist.tile([KROWS, M], BF16, tag="lhsw")
            featv = persist.tile([KROWS, P], BF16, tag="featv")
            featw = persist.tile([KROWS, P], BF16, tag="featw")
            # Pad rows [32:128] with zeros, column-chunked and scheduled so
            # chunk-0 pads land first: DVE takes featv ch0/1, ACT featw ch0/1,
            # GPSIMD (idle during compute) takes lhs + the ch2/3 tails in an
            # order that stays ahead of the compute loop.
            U32 = mybir.dt.uint32
            BASES = (KP, KP + 32, KP + 64)

            def pad(eng, t, b, c0, c1):
                if eng is nc.scalar:
                    eng.memzero(t[b : b + 32, c0:c1])
                else:
                    eng.memset(t[b : b + 32, c0:c1].bitcast(U32), 0)

            # chunk-0 pads spread over all three engines so featv/featw ch0
            # are ready fastest; GPSIMD takes lhs + the later-chunk tails
            pad(nc.vector, featv, KP, 0, CH)
            pad(nc.vector, featv, KP + 32, 0, CH)
            pad(nc.scalar, featw, KP, 0, CH)
            pad(nc.scalar, featw, KP + 32, 0, CH)
            pad(nc.gpsimd, lhsv, KP, 0, M)
            pad(nc.gpsimd, lhsv, KP + 32, 0, M)
            pad(nc.gpsimd, lhsv, KP + 64, 0, M)
            pad(nc.vector, featv, KP + 64, 0, CH)
            pad(nc.scalar, featw, KP + 64, 0, CH)
            for b in BASES:
                pad(nc.vector, featv, b, CH, 2 * CH)
                pad(nc.scalar, featw, b, CH, 2 * CH)
            for b in BASES:
                pad(nc.gpsimd, lhsw, b, 0, M)
            for ch in (2, 3):
                for b in BASES:
                    pad(nc.gpsimd, featv, b, ch * CH, (ch + 1) * CH)
                    pad(nc.gpsimd, featw, b, ch * CH, (ch + 1) * CH)
            # DMA order: what chunk 0 needs first
            nc.sync.dma_start(lhsv[0:KP, :], lhsv_d[:])
            c0 = slice(0, CH)
            nc.sync.dma_start(featv[0:KP, c0], featv_d[:, c0])
            nc.sync.dma_start(featw[0:KP, c0], featw_d[:, c0])
            nc.sync.dma_start(lhsw[0:KP, :], lhsw_d[:])
            for ch in range(1, NCH):
                sl = slice(ch * CH, (ch + 1) * CH)
                nc.sync.dma_start(featv[0:KP, sl], featv_d[:, sl])
                nc.sync.dma_start(featw[0:KP, sl], featw_d[:, sl])

            wmax = persist.tile([128, MT * NCH], F32, tag="wmax")
            wfin = persist.tile([128, MT], F32, tag="wfin")

            # 3 rotating chunk tiles (v -> |v| in-place -> -pairmin) + 1 dummy
            # target tile for clock-keeper matmuls: 4 * 4KB = all 8 PSUM banks.
            pp = tc.alloc_tile_pool(name="psum", bufs=3, space="PSUM")
            ppd = tc.alloc_tile_pool(name="psdum", bufs=1, space="PSUM")
            dummy = ppd.tile([128, CH], F32, tag="dum")
            DUMMIES = cfg.get("dummies", 2)

            def emit_v(ps, mt, ch):
                lv = lhsv[0:KREAL, mt * 128 : (mt + 1) * 128]
                for h in range(CH // 512):
                    off = ch * CH + h * 512
                    nc.tensor.matmul(
                        ps[:, h * 512 : h * 512 + 512], lv,
                        featv[0:KREAL, off : off + 512], start=True, stop=True,
                    )
                # |v| in place: WAR+WAW edges order the later accumulate
                nc.scalar.activation(ps[:], ps[:], AF.Abs)

            def emit_w(ps, mt, ch, ndum):
                lw = lhsw[0:KREAL, mt * 128 : (mt + 1) * 128]
                for h in range(CH // 512):
                    off = ch * CH + h * 512
                    nc.tensor.matmul(
                        ps[:, h * 512 : h * 512 + 512], lw,
                        featw[0:KREAL, off : off + 512], start=False, stop=True,
                        skip_group_check=True,
                    )
                # clock keepers: K=128 matmuls into the dummy tile keep the
                # PE at its boosted clock through consumer-bound stretches
                for dj in range(ndum):
                    half = (dj % 2) * 512
                    nc.tensor.matmul(
                        dummy[:, half : half + 512], lhsv[:, 0:128],
                        featv[:, 0:512], start=True, stop=True,
                    )

            def emit_tr(ps, mt, ch):
                nc.vector.tensor_reduce(
                    wmax[:, mt * NCH + ch : mt * NCH + ch + 1], ps[:],
                    AX.X, ALU.max,
                )

            # 3-stage software pipeline: v_k | w_{k-1} | tr_{k-2}. The TR
            # lags the accumulate by a full chunk so the DVE never waits on
            # the w-matmul semaphore (back-to-back TRs).
            chunks = [(mt, ch) for mt in range(MT) for ch in range(NCH)]
            tiles = {}
            k = 0
            for k, (mt, ch) in enumerate(chunks):
                ps_k = pp.tile([128, CH], F32, tag="ps")
                tiles[k] = ps_k
                emit_v(tiles[k], mt, ch)
                if k >= 1:
                    pm, pc = chunks[k - 1]
                    emit_w(tiles[k - 1], pm, pc, 1)
                if k >= 2:
                    qm, qc = chunks[k - 2]
                    emit_tr(tiles.pop(k - 2), qm, qc)
            emit_w(tiles[k], *chunks[k], 1)
            emit_tr(tiles.pop(k - 1), *chunks[k - 1])
            emit_tr(tiles.pop(k), *chunks[k])
            # combine chunk maxes: [128, MT, NCH] -> [128, MT]
            nc.vector.tensor_reduce(
                wfin[:], wmax[:].rearrange("p (a b) -> p a b", a=MT), AX.X, ALU.max
            )
            ppd.release()
            pp.release()
            nc.sync.dma_start(out_d[:], wfin[:])

    nc.compile()
    return nc


def _limb3(x):
    """bf16 3-way limb split of fp32 array: x ~= hi + mid + lo."""
    x = np.ascontiguousarray(x, np.float32)
    hi = x.astype(BF)
    r1 = x - hi.astype(np.float32)
    mid = r1.astype(BF)
    lo = (r1 - mid.astype(np.float32)).astype(BF)
    return hi, mid, lo


def _landmarks_cam(camera_pose, landmarks_3d):
    pose = np.asarray(camera_pose, np.float32)
    lm = np.asarray(landmarks_3d, np.float32)
    hom = np.concatenate([lm, np.ones((lm.shape[0], 1), np.float32)], axis=1)
    return (pose @ hom.T).T[:, :3].astype(np.float32)  # [M, 3]


def _fill_groups(dst, hi, mid, lo):
    """Rows 0-17: product groups {hh, hm, mh, mm, hl, lh} of a 3-dim factor."""
    dst[0:3] = hi
    dst[3:6] = mid
    dst[6:9] = hi
    dst[9:12] = mid
    dst[12:15] = lo
    dst[15:18] = hi


def _fill_groups_lhs(dst, hi, mid, lo):
    dst[0:3] = hi
    dst[3:6] = hi
    dst[6:9] = mid
    dst[9:12] = mid
    dst[12:15] = hi
    dst[15:18] = lo


def make_in_maps(cfg, splat_positions, camera_pose, landmarks_3d):
    C = cfg["n_cores"]
    S = cfg["s_per_core"]
    M = cfg["m_total"]
    P = S // 2
    sp = np.ascontiguousarray(np.asarray(splat_positions, np.float32))
    cam = _landmarks_cam(camera_pose, landmarks_3d)  # [M, 3]
    csq = np.sum(cam**2, axis=1, dtype=np.float32)  # [M]

    cT = cam.T  # [3, M]
    c_hi, c_mid, c_lo = _limb3(cT)
    q_hi, q_mid, q_lo = _limb3(-csq)

    lhsv = np.zeros((KP, M), BF)
    _fill_groups_lhs(lhsv, c_hi, c_mid, c_lo)
    lhsv[18:21] = np.float32(1.0)

    lhsw = np.zeros((KP, M), BF)
    _fill_groups_lhs(lhsw, c_hi, c_mid, c_lo)
    lhsw[18:21] = np.float32(1.0)
    lhsw[21] = q_hi
    lhsw[22] = q_mid
    lhsw[23] = q_lo

    maps = []
    for c in range(C):
        shard = sp[c * S : (c + 1) * S]  # [S, 3]
        a = shard[0::2]  # [P, 3]
        b = shard[1::2]
        sd = (b - a).T  # [3, P]  (v = c.(b-a) + sqdiff)
        ss = (a + b).T  # [3, P]  (-u = c.(a+b) - sqsum - csq)
        na = np.sum(a * a, axis=1, dtype=np.float32)
        nb = np.sum(b * b, axis=1, dtype=np.float32)
        sqdiff = (na - nb) * np.float32(0.5)  # [P]
        sqsum = (na + nb) * np.float32(-0.5)  # [P] (negated for -u)

        sd_hi, sd_mid, sd_lo = _limb3(sd)
        ss_hi, ss_mid, ss_lo = _limb3(ss)
        qd_hi, qd_mid, qd_lo = _limb3(sqdiff)
        qs_hi, qs_mid, qs_lo = _limb3(sqsum)

        featv = np.zeros((KP, P), BF)
        _fill_groups(featv, sd_hi, sd_mid, sd_lo)
        featv[18] = qd_hi
        featv[19] = qd_mid
        featv[20] = qd_lo

        featw = np.zeros((KP, P), BF)
        _fill_groups(featw, ss_hi, ss_mid, ss_lo)
        featw[18] = qs_hi
        featw[19] = qs_mid
        featw[20] = qs_lo
        featw[21:24] = np.float32(1.0)

        maps.append(
            {"featv": featv, "featw": featw, "lhsv": lhsv, "lhsw": lhsw}
        )
    return maps


_COMPILED = None


def _get_compiled():
    global _COMPILED
    if _COMPILED is None:
        _COMPILED = build(CFG)
    return _COMPILED


def kernel(
    splat_positions,
    camera_pose,
    landmarks_3d,
    landmarks_2d=None,
    camera_intrinsics=None,
    **_unused,
):
    nc = _get_compiled()
    in_maps = make_in_maps(CFG, splat_positions, camera_pose, landmarks_3d)
    core_ids = list(range(CFG["n_cores"]))
    try:
        res = run_bass_kernel_spmd(nc, in_maps, core_ids)
    except Exception:
        time.sleep(5.0)
        res = run_bass_kernel_spmd(nc, in_maps, core_ids)

    # negmin[p, mt] = -min over this core's splats of D^2 for landmark mt*128+p
    parts = np.stack([r["negmin"] for r in res.results], axis=0)  # [C, 128, MT]
    negmin = parts.max(axis=0)  # [128, MT]
    msq = np.maximum(-negmin, np.float32(0.0))  # [128, MT]
    MT = CFG["m_total"] // 128
    msq = msq.T.reshape(-1)  # m = mt*128 + p -> index [mt, p]
    d = np.sqrt(msq)
    valid = d < np.float32(1.0)
    num = np.int32(valid.sum())
    loss = np.float32(
        (msq * valid).sum(dtype=np.float32)
        / max(np.float32(3.0) * np.float32(num), np.float32(1.0))
    )
    meand = np.float32(
        (d * valid).sum(dtype=np.float32) / max(np.float32(num), np.float32(1.0))
    )
    return loss, num, meand


if __name__ == "__main__":
    build(CFG)
    print("build ok")
